# revision 46
# baseline (speedup 1.0000x reference)
"""Multi-head attention (B=4, S=2048, D=1024, H=16, dh=64, causal) on 8
Trainium2 NeuronCores.

Sharding: core (b, g) = batch b in 0..3, head-group g in 0..1 (8 heads each).
Each core computes attention for its 8 heads over its batch and a partial
output projection; the host sums the two head-group partials per batch and
adds the bias corrections (bo + bv @ Wo, since the V bias is not applied on
device -- attn rows sum to 1 so it folds into a constant row; the K bias is
dropped entirely because a per-query constant added to every score cancels
in softmax).

v4 layout (vs the v2 full-array baseline): every matmul runs in the 64x128
row-tiled PE mode (two 64-row tiles T0/T8), so the K=64 QK^T matmuls run as
concurrent head pairs at ~2x, and there are no tiling-mode switches anywhere
(a mode switch drains the PE, ~105ns, measured). Hardware constraint learned
the hard way: a PSUM bank written back-to-back by the two different tiles
faults the device (NRT_EXEC_UNIT_UNRECOVERABLE) because the second tile's
write window overlaps the first tile's drain. So every bank is fed by one
tile at a time:
  - heads are stored in pairs: qt/kt [128, 4, S] with partitions 0-63 = head
    2i and 64-127 = head 2i+1; scores for both heads of a pair compute
    concurrently (T0 -> psum bank 0, T8 -> bank 1), banks never shared
  - exp runs on the head pair jointly; causal masking is narrowed to the
    128-col diagonal window per k-block, split across gpsimd and vector
  - projections: per unit, T0 accumulates the lower contraction halves into
    bank0 while T8 accumulates the upper halves into bank1 (concurrent,
    disjoint banks); the eviction sums the two banks on the vector engine
  - output projection: same per-tile-bank trick, one 512-col half per phase
  - attn@V: phase1 accumulates [T0: e-lo(s0) -> po0 || T8: e-hi(s1) -> po1]
    per k-block in the current head-pair's stream; phase2 (the swapped
    halves) is carried into the NEXT head-pair's stream, so each po bank
    sees one contiguous T0 run, a well-separated handoff, then a T8 run.
    The softmax division happens once phase2 drains, in the next window.
  - per-chunk software pipeline as before: projections of chunk c+1 and
    output projections trail into the attention stream of chunk c
"""

import sys
import types

import numpy as np
import ml_dtypes


def _install_ntff_shim():
    """antenv.axon_hooks is absent in this image; recreate it and register the
    ctypes NTFF profile hook like trn_boot would, so trace=True works."""
    if "antenv.axon_hooks" in sys.modules:
        return
    mod = types.ModuleType("antenv.axon_hooks")
    state = {"hook": None}
    mod.set_axon_ntff_profile_hook = lambda h: state.__setitem__("hook", h)
    mod.get_axon_ntff_profile_hook = lambda: state["hook"]
    sys.modules["antenv.axon_hooks"] = mod
    try:
        import antenv

        antenv.axon_hooks = mod
    except ImportError:
        pass
    try:
        from trn_agent_boot.trn_boot import _ntff_profile_via_ctypes

        mod.set_axon_ntff_profile_hook(
            _ntff_profile_via_ctypes("/opt/axon/libaxon_pjrt.so")
        )
    except Exception:
        pass


_install_ntff_shim()

import concourse.bacc as bacc  # noqa: E402
import concourse.mybir as mybir  # noqa: E402
import concourse.tile as tile  # noqa: E402

P = 128
D = 1024
FG = 512  # features per core = 8 heads x 64
NH = 8  # heads per core
DH = 64
KC = D // P  # 8 contraction chunks for the projections
FC = FG // P  # 4 feature chunks of 128 (= head pairs)
TQ = 512  # q tile (free dim)
TK = 128  # k tile (partition dim)
F32 = mybir.dt.float32
BF16 = mybir.dt.bfloat16
AF = mybir.ActivationFunctionType
ADD = mybir.AluOpType.add


def build(tokens=2048, causal=True, variant=None):
    S = tokens
    NQC = S // TQ
    NKC = S // TK
    nc = bacc.Bacc()
    xt = nc.dram_tensor("XT", [D, S], BF16, kind="ExternalInput")
    wq = nc.dram_tensor("WQ", [D, FG], BF16, kind="ExternalInput")
    wk = nc.dram_tensor("WK", [D, FG], BF16, kind="ExternalInput")
    wv = nc.dram_tensor("WV", [D, FG], BF16, kind="ExternalInput")
    bq = nc.dram_tensor("BQ", [P, FC], F32, kind="ExternalInput")
    wo = nc.dram_tensor("WO", [P, FC, D], BF16, kind="ExternalInput")
    cm = nc.dram_tensor("CM", [P, TK], BF16, kind="ExternalInput")
    out = nc.dram_tensor("OUT", [S, D], BF16, kind="ExternalOutput")

    with tile.TileContext(nc) as tc, nc.allow_low_precision(
        reason="bf16 matmul inputs"
    ):
        with tc.tile_pool(name="const", bufs=1) as cpool, tc.tile_pool(
            name="qkv", bufs=1
        ) as qkv, tc.tile_pool(name="w", bufs=1) as wpool, tc.tile_pool(
            name="xt", bufs=2
        ) as xpool, tc.tile_pool(name="e", bufs=16) as epool, tc.tile_pool(
            name="r", bufs=4
        ) as rpool, tc.tile_pool(name="o", bufs=3) as opool, tc.tile_pool(
            name="pss", bufs=2, space="PSUM"
        ) as pss, tc.tile_pool(
            name="pso", bufs=1, space="PSUM"
        ) as pso, tc.tile_pool(name="pj", bufs=2, space="PSUM") as pjp:
            # ---- DMAs split across the two hardware DGE rings (sync +
            # scalar); the first chunk's X and Wq are split per-kc so the
            # first projection matmuls can start as soon as each 128-row
            # slice lands (Tile tracks subtile deps) ----
            bq_sb = cpool.tile([P, FC], F32, name="bq_sb")
            nc.sync.dma_start(bq_sb[:], bq[:])

            wq_sb = wpool.tile([P, KC, FG], BF16, name="wq_sb")
            wk_sb = wpool.tile([P, KC, FG], BF16, name="wk_sb")
            wv_sb = wpool.tile([P, KC, FG], BF16, name="wv_sb")

            xts = [None] * NQC

            def fetch_x(c):
                xts[c] = xpool.tile([P, KC, TQ], BF16, tag="xt", name="xt_t")
                if c == 0:
                    for kc in range(KC):
                        nc.sync.dma_start(
                            xts[c][:, kc, :],
                            xt[kc * P : (kc + 1) * P, 0:TQ],
                        )
                        nc.scalar.dma_start(
                            wq_sb[:, kc, :], wq[kc * P : (kc + 1) * P, :]
                        )
                else:
                    nc.sync.dma_start(
                        xts[c][:],
                        xt[:, c * TQ : (c + 1) * TQ].rearrange(
                            "(kc p) t -> p kc t", p=P
                        ),
                    )

            fetch_x(0)
            nc.scalar.dma_start(
                wk_sb[:], wk.rearrange("(kc p) m -> p kc m", p=P)
            )
            nc.sync.dma_start(
                wv_sb[:], wv.rearrange("(kc p) m -> p kc m", p=P)
            )
            cm_sb = cpool.tile([P, TK], BF16, name="cm_sb")
            nc.scalar.dma_start(cm_sb[:], cm[:])

            # head-pair layouts: partitions 0-63 = head 2i, 64-127 = head 2i+1
            qt_sb = qkv.tile([P, FC, S], BF16, name="qt_sb")
            kt_sb = qkv.tile([P, FC, S], BF16, name="kt_sb")
            v_sb = qkv.tile([P, NKC, NH, 2 * DH], BF16, name="v_sb")
            u_sb = qkv.tile([P, FC, S], BF16, name="u_sb")

            wo_sb = wpool.tile([P, FC, D], BF16, name="wo_sb")

            # ---- projection emission (full-array matmuls, one psum
            # bank per unit, double-buffered) ----
            UNITS = [
                ("q", 0), ("q", 1), ("q", 2), ("q", 3),
                ("k", 0), ("k", 1), ("k", 2), ("k", 3),
                ("v", 0), ("v", 1), ("v", 2), ("v", 3),
            ]

            def emit_unit(c, u):
                kind, idx = UNITS[u]
                tsl = slice(c * TQ, (c + 1) * TQ)
                if kind in ("q", "k"):
                    ps = pjp.tile([P, TQ], F32, tag="pj", name="ps_qk")
                    w_sb = wq_sb if kind == "q" else wk_sb
                    dst = qt_sb if kind == "q" else kt_sb
                    for kc in range(KC):
                        nc.tensor.matmul(
                            ps[:],
                            w_sb[:, kc, idx * P : (idx + 1) * P],
                            xts[c][:, kc, :],
                            start=(kc == 0),
                            stop=(kc == KC - 1),
                            skip_group_check=True,
                        )
                    if kind == "q":
                        nc.vector.tensor_tensor(
                            dst[:, idx, tsl],
                            ps[:],
                            bq_sb[:, idx : idx + 1].to_broadcast([P, TQ]),
                            ADD,
                        )
                    else:
                        nc.vector.tensor_copy(dst[:, idx, tsl], ps[:])
                else:
                    ps = pjp.tile([P, NH, DH], F32, tag="pj", name="ps_v")
                    for kc in range(KC):
                        nc.tensor.matmul(
                            ps[:],
                            xts[c][:, kc, idx * P : (idx + 1) * P],
                            wv_sb[:, kc, :],
                            start=(kc == 0),
                            stop=(kc == KC - 1),
                            skip_group_check=True,
                        )
                    tg = c * (TQ // P) + idx
                    nc.vector.tensor_copy(v_sb[:, tg, :, DH : 2 * DH], ps[:])

            # ---- attn@V: full-array (128-row) matmuls, emitted in batches
            # of G blocks so the 64<->128 tiling-mode switch cost (~105ns,
            # PE drain) is amortized: 2 switches per G blocks instead of 2
            # per block. rows 0-63 of po[s] hold Z replicated (ones block
            # in v_sb); the division runs right after the window's last
            # batch ----
            AV_G = 4

            def flush_avq(avq, po, hp, last):
                for i, (e_t, kc, c0) in enumerate(avq):
                    fin = last and i == len(avq) - 1
                    for s in range(2):
                        nc.tensor.matmul(
                            po[:, s, c0:],
                            v_sb[:, kc, 2 * hp + s, :],
                            e_t[:, s, c0:],
                            start=(kc == 0),
                            stop=(fin and s == 1),
                            skip_group_check=True,
                        )
                avq.clear()

            def emit_div(po, hp, qc):
                qtsl = slice(qc * TQ, (qc + 1) * TQ)
                for s in range(2):
                    rb = rpool.tile([DH, TQ], F32, tag="rb", name="rb_t")
                    nc.vector.reciprocal_approx_fast(rb[:], po[0:DH, s, :])
                    nc.vector.tensor_mul(
                        u_sb[s * DH : (s + 1) * DH, hp, qtsl],
                        po[DH:P, s, :],
                        rb[:],
                    )

            def attn(hp, qc, pump, fillpoint, pend):
                nblocks = 4 * (qc + 1) if causal else NKC
                po = pso.tile([P, 2, TQ], F32, tag="po", name="po")
                avq = []
                for kc in range(nblocks):
                    j = kc - 4 * qc
                    c0 = TK * j if (causal and j >= 0) else 0
                    ps = pss.tile([P, 2, TQ], F32, tag="ps", name="ps_s")
                    e_t = epool.tile([P, 2, TQ], BF16, tag="e", name="e_t")
                    for s in range(2):
                        pb = DH * s
                        nc.tensor.matmul(
                            ps[:, s, c0:],
                            kt_sb[pb : pb + DH, hp, kc * TK : (kc + 1) * TK],
                            qt_sb[pb : pb + DH, hp, qc * TQ + c0 : (qc + 1) * TQ],
                            start=True,
                            stop=True,
                        )
                    nc.scalar.activation(
                        e_t[:, :, c0:], ps[:, :, c0:], AF.Exp, scale=0.125
                    )
                    if causal and j >= 0:
                        # only the 128-col diagonal window needs masking; the
                        # attn@V matmul skips cols < c0 and cols beyond the
                        # window are fully valid
                        for s, eng in ((0, nc.gpsimd), (1, nc.vector)):
                            eng.tensor_mul(
                                e_t[:, s, c0 : c0 + TK],
                                e_t[:, s, c0 : c0 + TK],
                                cm_sb[:],
                            )
                    avq.append((e_t, kc, c0))
                    pump()
                    if pend and kc == 1:
                        # the previous window's tail batch + division, now
                        # that its last exp+mask have long completed
                        ppo, php, pqc, pavq = pend.pop()
                        flush_avq(pavq, ppo, php, last=True)
                        emit_div(ppo, php, pqc)
                        # fill lands inside the 128-mode segment
                        fillpoint()
                    if len(avq) >= 2 * AV_G:
                        head = avq[:AV_G]
                        del avq[:AV_G]
                        flush_avq(head, po, hp, last=False)
                        fillpoint()
                # defer the tail batch + division into the next window
                pend.append((po, hp, qc, avq[:]))

            # ---- output projection for one 128-token block, one 512-col
            # half per call (phase ph in {0, 1}) ----
            def outproj_slice(qc, t8, ph, final=False):
                tg = qc * (TQ // P) + t8
                if ph == 0:
                    o_t = opool.tile([P, D], BF16, tag="o", name="o_t")
                    outproj_slice.cur[tg] = o_t
                else:
                    o_t = outproj_slice.cur.pop(tg)
                ps = pjp.tile([P, 512], F32, tag="pj", name="ps_o")
                for i in range(FC):
                    nc.tensor.matmul(
                        ps[:],
                        u_sb[:, i, tg * P : (tg + 1) * P],
                        wo_sb[:, i, ph * 512 : (ph + 1) * 512],
                        start=(i == 0),
                        stop=(i == FC - 1),
                        skip_group_check=True,
                    )
                osl = o_t[:, ph * 512 : (ph + 1) * 512]
                if final and ph == 0:
                    # split the drain-critical evictions across the
                    # then-idle scalar engine and the vector engine
                    nc.scalar.activation(osl, ps[:], AF.Copy)
                else:
                    nc.vector.tensor_copy(osl, ps[:])
                if ph == 1:
                    deng = nc.scalar if final else nc.sync
                    deng.dma_start(out[tg * P : (tg + 1) * P, :], o_t[:])

            outproj_slice.cur = {}

            # ---- schedule ----
            # warm the PE clock (HAM ramps to 2.4GHz after ~4us of continuous
            # matmul activity) with dummy 64-mode matmul pairs on a scratch
            # tile while the input DMAs land
            warm_sb = cpool.tile([P, TQ], BF16, name="warm_sb")
            nc.gpsimd.memset(warm_sb[:], 0.0)
            # ones block for the softmax-denominator rows of attn@V
            nc.gpsimd.memset(v_sb[:, :, :, 0:DH], 1.0)
            for _ in range(30):
                wps = pjp.tile([P, TQ], F32, tag="pj", name="ps_warm")
                nc.tensor.matmul(
                    wps[:], warm_sb[:, 0:P], warm_sb[:], start=True, stop=True
                )
            # chunk 0 runs Q then K then V units, matching DMA arrival order
            for u in range(len(UNITS)):
                emit_unit(0, u)

            def interleave(a, b):
                # proportional merge of two thunk lists
                res = []
                ia = ib = 0
                n = len(a) + len(b)
                for _ in range(n):
                    if ib >= len(b) or (
                        ia < len(a) and ia * len(b) <= ib * len(a)
                    ):
                        res.append(a[ia])
                        ia += 1
                    else:
                        res.append(b[ib])
                        ib += 1
                return res

            pend = []
            for qc in range(NQC):
                if qc + 1 < NQC:
                    fetch_x(qc + 1)
                if qc == 0:
                    nc.sync.dma_start(wo_sb[:], wo[:])
                units = (
                    [
                        (lambda u=u, c=qc + 1: emit_unit(c, u))
                        for u in range(len(UNITS))
                    ]
                    if qc + 1 < NQC
                    else []
                )
                oproj = []
                if qc == 3:
                    # all non-final output projections run here: the qc3
                    # windows have no projection units, and the attention
                    # inner loop alone underruns the exp-bound block rate
                    oproj = [
                        (lambda o=o, t=t, ph=ph: outproj_slice(o, t, ph))
                        for o in range(3)
                        for t in range(4)
                        for ph in range(2)
                    ]
                fill = interleave(units, oproj)
                nblocks = 4 * (qc + 1) if causal else NKC
                blocks_total = FC * nblocks
                state = [0, 0]  # blocks done, fill items emitted

                def pump():
                    state[0] += 1

                def fillpoint():
                    tgt = len(fill) * state[0] // blocks_total
                    while state[1] < tgt:
                        fill[state[1]]()
                        state[1] += 1

                for hp in range(FC):
                    attn(hp, qc, pump, fillpoint, pend)
                while state[1] < len(fill):
                    fill[state[1]]()
                    state[1] += 1
                if qc == NQC - 1:
                    while pend:
                        ppo, php, pqc, pavq = pend.pop(0)
                        flush_avq(pavq, ppo, php, last=True)
                        emit_div(ppo, php, pqc)
                    for t8 in range(TQ // P):
                        outproj_slice(qc, t8, 0, final=True)
                        outproj_slice(qc, t8, 1, final=True)

    nc.compile()
    return nc


def make_in_maps(X, Wq, bq, Wk, Wv, Wo, causal):
    bf = ml_dtypes.bfloat16
    # cm[p, g] = 1.0 where k-position p of a diagonal 128-block may attend
    # to q-position g of the same 128-block: p <= g
    cmv = (np.arange(P)[:, None] <= np.arange(TK)[None, :]).astype(bf)
    in_maps = []
    for b in range(4):
        for g in range(2):
            sl = slice(g * FG, (g + 1) * FG)
            bq2 = np.ascontiguousarray(
                bq[sl].reshape(FC, 2, DH).transpose(1, 2, 0).reshape(P, FC)
            ).astype(np.float32)
            in_maps.append(
                {
                    "XT": np.ascontiguousarray(X[b].T).astype(bf),
                    "WQ": np.ascontiguousarray(Wq[:, sl]).astype(bf),
                    "WK": np.ascontiguousarray(Wk[:, sl]).astype(bf),
                    "WV": np.ascontiguousarray(Wv[:, sl]).astype(bf),
                    "BQ": bq2,
                    "WO": np.ascontiguousarray(
                        Wo[sl, :].reshape(FC, P, D).transpose(1, 0, 2)
                    ).astype(bf),
                    "CM": cmv,
                }
            )
    return in_maps


_CACHE = {}


def _get_program(causal):
    key = bool(causal)
    if key not in _CACHE:
        _CACHE[key] = build(tokens=2048, causal=key)
    return _CACHE[key]


def kernel(X, Wq, bq, Wk, bk, Wv, bv, Wo, bo, causal, **_unused):
    from concourse.bass_utils import run_bass_kernel_spmd

    X = np.asarray(X, np.float32)
    Wq, bq = np.asarray(Wq, np.float32), np.asarray(bq, np.float32)
    Wk = np.asarray(Wk, np.float32)
    Wv = np.asarray(Wv, np.float32)
    Wo, bo = np.asarray(Wo, np.float32), np.asarray(bo, np.float32)
    bv = np.asarray(bv, np.float32)
    causal_flag = bool(np.asarray(causal).item())

    nc = _get_program(causal_flag)
    in_maps = make_in_maps(X, Wq, bq, Wk, Wv, Wo, causal_flag)
    res = run_bass_kernel_spmd(nc, in_maps, core_ids=list(range(8)))

    # attn rows sum to 1, so the missing V bias contributes bv @ Wo exactly
    corr = bv @ Wo + bo
    outs = []
    for b in range(4):
        o = (
            res.results[2 * b]["OUT"].astype(np.float32)
            + res.results[2 * b + 1]["OUT"].astype(np.float32)
            + corr
        )
        outs.append(o)
    return np.stack(outs).astype(np.float32)


# revision 47
# speedup vs baseline: 1.0024x; 1.0024x over previous
"""Multi-head attention (B=4, S=2048, D=1024, H=16, dh=64, causal) on 8
Trainium2 NeuronCores.

Sharding: core (b, g) = batch b in 0..3, head-group g in 0..1 (8 heads each).
Each core computes attention for its 8 heads over its batch and a partial
output projection; the host sums the two head-group partials per batch and
adds the bias corrections (bo + bv @ Wo, since the V bias is not applied on
device -- attn rows sum to 1 so it folds into a constant row; the K bias is
dropped entirely because a per-query constant added to every score cancels
in softmax).

v7 (vs the v2 full-array baseline, 293.7us -> ~256us):
  - the K=64 QK^T scores matmuls run in the 64x128 row-tiled PE mode as
    CONCURRENT head pairs (T0 computes head 2i from partitions 0-63 into
    psum bank 0, T8 head 2i+1 from partitions 64-127 into bank 1), halving
    their PE time. qt/kt/u use the head-pair layout [128, 4, S].
    Hardware rules learned en route: two row tiles must never write the
    same psum bank in adjacent streams (the second tile's writes overlap
    the first tile's drain -> NRT_EXEC_UNIT_UNRECOVERABLE), and a tiling-
    mode switch costs a ~105ns PE drain, so:
  - attn@V stays full-array (K=128) but is emitted in BATCHES of AV_G
    blocks with a one-batch lag, so the 64<->128 mode switch cost is paid
    once per batch and every entry's exp+mask has already landed; each
    window's tail batch + softmax division are deferred into the next
    window so the PE never stalls on the freshest exp
  - exp runs on head pairs [128, 2, 512] on the scalar engine; causal
    masking is narrowed to the 128-col diagonal window per k-block and
    split across gpsimd (s=0) and vector (s=1); scores/exp/attn@V all
    skip the fully-masked column range of diagonal blocks (c0 = 128*j)
  - projections/output projection/warmup keep the baseline full-array
    single-bank form (pjp double-buffered, single-op evictions)
  - per-chunk software pipeline: projections of chunk c+1 fill the
    attention windows of chunk c; ALL non-final output projections fill
    the qc=3 windows (which otherwise underrun the exp-bound block rate),
    emitted at 128-mode batch boundaries to avoid extra mode switches
  - output stored bf16; host upcasts and sums the two head-group partials
"""

import sys
import types

import numpy as np
import ml_dtypes


def _install_ntff_shim():
    """antenv.axon_hooks is absent in this image; recreate it and register the
    ctypes NTFF profile hook like trn_boot would, so trace=True works."""
    if "antenv.axon_hooks" in sys.modules:
        return
    mod = types.ModuleType("antenv.axon_hooks")
    state = {"hook": None}
    mod.set_axon_ntff_profile_hook = lambda h: state.__setitem__("hook", h)
    mod.get_axon_ntff_profile_hook = lambda: state["hook"]
    sys.modules["antenv.axon_hooks"] = mod
    try:
        import antenv

        antenv.axon_hooks = mod
    except ImportError:
        pass
    try:
        from trn_agent_boot.trn_boot import _ntff_profile_via_ctypes

        mod.set_axon_ntff_profile_hook(
            _ntff_profile_via_ctypes("/opt/axon/libaxon_pjrt.so")
        )
    except Exception:
        pass


_install_ntff_shim()

import concourse.bacc as bacc  # noqa: E402
import concourse.mybir as mybir  # noqa: E402
import concourse.tile as tile  # noqa: E402

P = 128
D = 1024
FG = 512  # features per core = 8 heads x 64
NH = 8  # heads per core
DH = 64
KC = D // P  # 8 contraction chunks for the projections
FC = FG // P  # 4 feature chunks of 128 (= head pairs)
TQ = 512  # q tile (free dim)
TK = 128  # k tile (partition dim)
F32 = mybir.dt.float32
BF16 = mybir.dt.bfloat16
AF = mybir.ActivationFunctionType
ADD = mybir.AluOpType.add


def build(tokens=2048, causal=True, variant=None):
    S = tokens
    NQC = S // TQ
    NKC = S // TK
    nc = bacc.Bacc()
    xt = nc.dram_tensor("XT", [D, S], BF16, kind="ExternalInput")
    wq = nc.dram_tensor("WQ", [D, FG], BF16, kind="ExternalInput")
    wk = nc.dram_tensor("WK", [D, FG], BF16, kind="ExternalInput")
    wv = nc.dram_tensor("WV", [D, FG], BF16, kind="ExternalInput")
    bq = nc.dram_tensor("BQ", [P, FC], F32, kind="ExternalInput")
    wo = nc.dram_tensor("WO", [P, FC, D], BF16, kind="ExternalInput")
    cm = nc.dram_tensor("CM", [P, TK], BF16, kind="ExternalInput")
    out = nc.dram_tensor("OUT", [S, D], BF16, kind="ExternalOutput")

    with tile.TileContext(nc) as tc, nc.allow_low_precision(
        reason="bf16 matmul inputs"
    ):
        with tc.tile_pool(name="const", bufs=1) as cpool, tc.tile_pool(
            name="qkv", bufs=1
        ) as qkv, tc.tile_pool(name="w", bufs=1) as wpool, tc.tile_pool(
            name="xt", bufs=2
        ) as xpool, tc.tile_pool(name="e", bufs=16) as epool, tc.tile_pool(
            name="r", bufs=4
        ) as rpool, tc.tile_pool(name="o", bufs=3) as opool, tc.tile_pool(
            name="pss", bufs=2, space="PSUM"
        ) as pss, tc.tile_pool(
            name="pso", bufs=1, space="PSUM"
        ) as pso, tc.tile_pool(name="pj", bufs=2, space="PSUM") as pjp:
            # ---- DMAs split across the two hardware DGE rings (sync +
            # scalar); the first chunk's X and Wq are split per-kc so the
            # first projection matmuls can start as soon as each 128-row
            # slice lands (Tile tracks subtile deps) ----
            bq_sb = cpool.tile([P, FC], F32, name="bq_sb")
            nc.sync.dma_start(bq_sb[:], bq[:])

            wq_sb = wpool.tile([P, KC, FG], BF16, name="wq_sb")
            wk_sb = wpool.tile([P, KC, FG], BF16, name="wk_sb")
            wv_sb = wpool.tile([P, KC, FG], BF16, name="wv_sb")

            xts = [None] * NQC

            def fetch_x(c):
                xts[c] = xpool.tile([P, KC, TQ], BF16, tag="xt", name="xt_t")
                if c == 0:
                    for kc in range(KC):
                        nc.sync.dma_start(
                            xts[c][:, kc, :],
                            xt[kc * P : (kc + 1) * P, 0:TQ],
                        )
                        nc.scalar.dma_start(
                            wq_sb[:, kc, :], wq[kc * P : (kc + 1) * P, :]
                        )
                else:
                    nc.sync.dma_start(
                        xts[c][:],
                        xt[:, c * TQ : (c + 1) * TQ].rearrange(
                            "(kc p) t -> p kc t", p=P
                        ),
                    )

            fetch_x(0)
            nc.scalar.dma_start(
                wk_sb[:], wk.rearrange("(kc p) m -> p kc m", p=P)
            )
            nc.sync.dma_start(
                wv_sb[:], wv.rearrange("(kc p) m -> p kc m", p=P)
            )
            cm_sb = cpool.tile([P, TK], BF16, name="cm_sb")
            nc.scalar.dma_start(cm_sb[:], cm[:])

            # head-pair layouts: partitions 0-63 = head 2i, 64-127 = head 2i+1
            qt_sb = qkv.tile([P, FC, S], BF16, name="qt_sb")
            kt_sb = qkv.tile([P, FC, S], BF16, name="kt_sb")
            v_sb = qkv.tile([P, NKC, NH, 2 * DH], BF16, name="v_sb")
            u_sb = qkv.tile([P, FC, S], BF16, name="u_sb")

            wo_sb = wpool.tile([P, FC, D], BF16, name="wo_sb")

            # ---- projection emission (full-array matmuls, one psum
            # bank per unit, double-buffered) ----
            UNITS = [
                ("q", 0), ("q", 1), ("q", 2), ("q", 3),
                ("k", 0), ("k", 1), ("k", 2), ("k", 3),
                ("v", 0), ("v", 1), ("v", 2), ("v", 3),
            ]

            def emit_unit(c, u):
                kind, idx = UNITS[u]
                tsl = slice(c * TQ, (c + 1) * TQ)
                if kind in ("q", "k"):
                    ps = pjp.tile([P, TQ], F32, tag="pj", name="ps_qk")
                    w_sb = wq_sb if kind == "q" else wk_sb
                    dst = qt_sb if kind == "q" else kt_sb
                    for kc in range(KC):
                        nc.tensor.matmul(
                            ps[:],
                            w_sb[:, kc, idx * P : (idx + 1) * P],
                            xts[c][:, kc, :],
                            start=(kc == 0),
                            stop=(kc == KC - 1),
                            skip_group_check=True,
                        )
                    if kind == "q":
                        nc.vector.tensor_tensor(
                            dst[:, idx, tsl],
                            ps[:],
                            bq_sb[:, idx : idx + 1].to_broadcast([P, TQ]),
                            ADD,
                        )
                    else:
                        nc.vector.tensor_copy(dst[:, idx, tsl], ps[:])
                else:
                    ps = pjp.tile([P, NH, DH], F32, tag="pj", name="ps_v")
                    for kc in range(KC):
                        nc.tensor.matmul(
                            ps[:],
                            xts[c][:, kc, idx * P : (idx + 1) * P],
                            wv_sb[:, kc, :],
                            start=(kc == 0),
                            stop=(kc == KC - 1),
                            skip_group_check=True,
                        )
                    tg = c * (TQ // P) + idx
                    nc.vector.tensor_copy(v_sb[:, tg, :, DH : 2 * DH], ps[:])

            # ---- attn@V: full-array (128-row) matmuls, emitted in batches
            # of G blocks so the 64<->128 tiling-mode switch cost (~105ns,
            # PE drain) is amortized: 2 switches per G blocks instead of 2
            # per block. rows 0-63 of po[s] hold Z replicated (ones block
            # in v_sb); the division runs right after the window's last
            # batch ----
            AV_G = 4

            def flush_avq(avq, po, hp, last):
                for i, (e_t, kc, c0) in enumerate(avq):
                    fin = last and i == len(avq) - 1
                    for s in range(2):
                        nc.tensor.matmul(
                            po[:, s, c0:],
                            v_sb[:, kc, 2 * hp + s, :],
                            e_t[:, s, c0:],
                            start=(kc == 0),
                            stop=(fin and s == 1),
                            skip_group_check=True,
                        )
                avq.clear()

            def emit_div(po, hp, qc):
                qtsl = slice(qc * TQ, (qc + 1) * TQ)
                for s in range(2):
                    rb = rpool.tile([DH, TQ], F32, tag="rb", name="rb_t")
                    nc.vector.reciprocal_approx_fast(rb[:], po[0:DH, s, :])
                    nc.vector.tensor_mul(
                        u_sb[s * DH : (s + 1) * DH, hp, qtsl],
                        po[DH:P, s, :],
                        rb[:],
                    )

            def attn(hp, qc, pump, fillpoint, pend):
                nblocks = 4 * (qc + 1) if causal else NKC
                po = pso.tile([P, 2, TQ], F32, tag="po", name="po")
                avq = []
                for kc in range(nblocks):
                    j = kc - 4 * qc
                    c0 = TK * j if (causal and j >= 0) else 0
                    ps = pss.tile([P, 2, TQ], F32, tag="ps", name="ps_s")
                    e_t = epool.tile([P, 2, TQ], BF16, tag="e", name="e_t")
                    for s in range(2):
                        pb = DH * s
                        nc.tensor.matmul(
                            ps[:, s, c0:],
                            kt_sb[pb : pb + DH, hp, kc * TK : (kc + 1) * TK],
                            qt_sb[pb : pb + DH, hp, qc * TQ + c0 : (qc + 1) * TQ],
                            start=True,
                            stop=True,
                        )
                    nc.scalar.activation(
                        e_t[:, :, c0:], ps[:, :, c0:], AF.Exp, scale=0.125
                    )
                    if causal and j >= 0:
                        # only the 128-col diagonal window needs masking; the
                        # attn@V matmul skips cols < c0 and cols beyond the
                        # window are fully valid
                        for s, eng in ((0, nc.gpsimd), (1, nc.vector)):
                            eng.tensor_mul(
                                e_t[:, s, c0 : c0 + TK],
                                e_t[:, s, c0 : c0 + TK],
                                cm_sb[:],
                            )
                    avq.append((e_t, kc, c0))
                    pump()
                    if pend and kc == 1:
                        # the previous window's tail batch + division, now
                        # that its last exp+mask have long completed
                        ppo, php, pqc, pavq = pend.pop()
                        flush_avq(pavq, ppo, php, last=True)
                        emit_div(ppo, php, pqc)
                        # fill lands inside the 128-mode segment
                        fillpoint()
                    if len(avq) >= 2 * AV_G:
                        head = avq[:AV_G]
                        del avq[:AV_G]
                        flush_avq(head, po, hp, last=False)
                        fillpoint()
                # defer the tail batch + division into the next window
                pend.append((po, hp, qc, avq[:]))

            # ---- output projection for one 128-token block, one 512-col
            # half per call (phase ph in {0, 1}) ----
            def outproj_slice(qc, t8, ph, final=False):
                tg = qc * (TQ // P) + t8
                if ph == 0:
                    o_t = opool.tile([P, D], BF16, tag="o", name="o_t")
                    outproj_slice.cur[tg] = o_t
                else:
                    o_t = outproj_slice.cur.pop(tg)
                ps = pjp.tile([P, 512], F32, tag="pj", name="ps_o")
                for i in range(FC):
                    nc.tensor.matmul(
                        ps[:],
                        u_sb[:, i, tg * P : (tg + 1) * P],
                        wo_sb[:, i, ph * 512 : (ph + 1) * 512],
                        start=(i == 0),
                        stop=(i == FC - 1),
                        skip_group_check=True,
                    )
                osl = o_t[:, ph * 512 : (ph + 1) * 512]
                if final and ph == 0:
                    # split the drain-critical evictions across the
                    # then-idle scalar engine and the vector engine
                    nc.scalar.activation(osl, ps[:], AF.Copy)
                else:
                    nc.vector.tensor_copy(osl, ps[:])
                if ph == 1:
                    deng = nc.scalar if final else nc.sync
                    deng.dma_start(out[tg * P : (tg + 1) * P, :], o_t[:])

            outproj_slice.cur = {}

            # ---- schedule ----
            # warm the PE clock (HAM ramps to 2.4GHz after ~4us of continuous
            # matmul activity) with dummy 64-mode matmul pairs on a scratch
            # tile while the input DMAs land
            warm_sb = cpool.tile([P, TQ], BF16, name="warm_sb")
            nc.gpsimd.memset(warm_sb[:], 0.0)
            # ones block for the softmax-denominator rows of attn@V
            nc.gpsimd.memset(v_sb[:, :, :, 0:DH], 1.0)
            for _ in range(30):
                wps = pjp.tile([P, TQ], F32, tag="pj", name="ps_warm")
                nc.tensor.matmul(
                    wps[:], warm_sb[:, 0:P], warm_sb[:], start=True, stop=True
                )
            # chunk 0 runs Q then K then V units, matching DMA arrival order
            for u in range(len(UNITS)):
                emit_unit(0, u)

            def interleave(a, b):
                # proportional merge of two thunk lists
                res = []
                ia = ib = 0
                n = len(a) + len(b)
                for _ in range(n):
                    if ib >= len(b) or (
                        ia < len(a) and ia * len(b) <= ib * len(a)
                    ):
                        res.append(a[ia])
                        ia += 1
                    else:
                        res.append(b[ib])
                        ib += 1
                return res

            pend = []
            for qc in range(NQC):
                if qc + 1 < NQC:
                    fetch_x(qc + 1)
                if qc == 0:
                    nc.sync.dma_start(wo_sb[:], wo[:])
                units = (
                    [
                        (lambda u=u, c=qc + 1: emit_unit(c, u))
                        for u in range(len(UNITS))
                    ]
                    if qc + 1 < NQC
                    else []
                )
                oproj = []
                if qc == 3:
                    # all non-final output projections run here: the qc3
                    # windows have no projection units, and the attention
                    # inner loop alone underruns the exp-bound block rate
                    oproj = [
                        (lambda o=o, t=t, ph=ph: outproj_slice(o, t, ph))
                        for o in range(3)
                        for t in range(4)
                        for ph in range(2)
                    ]
                fill = interleave(units, oproj)
                nblocks = 4 * (qc + 1) if causal else NKC
                blocks_total = FC * nblocks
                state = [0, 0]  # blocks done, fill items emitted

                def pump():
                    state[0] += 1

                def fillpoint():
                    tgt = len(fill) * state[0] // blocks_total
                    while state[1] < tgt:
                        fill[state[1]]()
                        state[1] += 1

                for hp in range(FC):
                    attn(hp, qc, pump, fillpoint, pend)
                while state[1] < len(fill):
                    fill[state[1]]()
                    state[1] += 1
                if qc == NQC - 1:
                    while pend:
                        ppo, php, pqc, pavq = pend.pop(0)
                        flush_avq(pavq, ppo, php, last=True)
                        emit_div(ppo, php, pqc)
                    for t8 in range(TQ // P):
                        outproj_slice(qc, t8, 0, final=True)
                        outproj_slice(qc, t8, 1, final=True)

    nc.compile()
    return nc


def make_in_maps(X, Wq, bq, Wk, Wv, Wo, causal):
    bf = ml_dtypes.bfloat16
    # cm[p, g] = 1.0 where k-position p of a diagonal 128-block may attend
    # to q-position g of the same 128-block: p <= g
    cmv = (np.arange(P)[:, None] <= np.arange(TK)[None, :]).astype(bf)
    in_maps = []
    for b in range(4):
        for g in range(2):
            sl = slice(g * FG, (g + 1) * FG)
            bq2 = np.ascontiguousarray(
                bq[sl].reshape(FC, 2, DH).transpose(1, 2, 0).reshape(P, FC)
            ).astype(np.float32)
            in_maps.append(
                {
                    "XT": np.ascontiguousarray(X[b].T).astype(bf),
                    "WQ": np.ascontiguousarray(Wq[:, sl]).astype(bf),
                    "WK": np.ascontiguousarray(Wk[:, sl]).astype(bf),
                    "WV": np.ascontiguousarray(Wv[:, sl]).astype(bf),
                    "BQ": bq2,
                    "WO": np.ascontiguousarray(
                        Wo[sl, :].reshape(FC, P, D).transpose(1, 0, 2)
                    ).astype(bf),
                    "CM": cmv,
                }
            )
    return in_maps


_CACHE = {}


def _get_program(causal):
    key = bool(causal)
    if key not in _CACHE:
        _CACHE[key] = build(tokens=2048, causal=key)
    return _CACHE[key]


def kernel(X, Wq, bq, Wk, bk, Wv, bv, Wo, bo, causal, **_unused):
    from concourse.bass_utils import run_bass_kernel_spmd

    X = np.asarray(X, np.float32)
    Wq, bq = np.asarray(Wq, np.float32), np.asarray(bq, np.float32)
    Wk = np.asarray(Wk, np.float32)
    Wv = np.asarray(Wv, np.float32)
    Wo, bo = np.asarray(Wo, np.float32), np.asarray(bo, np.float32)
    bv = np.asarray(bv, np.float32)
    causal_flag = bool(np.asarray(causal).item())

    nc = _get_program(causal_flag)
    in_maps = make_in_maps(X, Wq, bq, Wk, Wv, Wo, causal_flag)
    res = run_bass_kernel_spmd(nc, in_maps, core_ids=list(range(8)))

    # attn rows sum to 1, so the missing V bias contributes bv @ Wo exactly
    corr = bv @ Wo + bo
    outs = []
    for b in range(4):
        o = (
            res.results[2 * b]["OUT"].astype(np.float32)
            + res.results[2 * b + 1]["OUT"].astype(np.float32)
            + corr
        )
        outs.append(o)
    return np.stack(outs).astype(np.float32)
